# revision 6
# baseline (speedup 1.0000x reference)
"""Trainium2 Bass kernel for nn_MultiLayerPerceptron_he_36412732735948.

GCN + MLP on B=32 point clouds of N=1024 nodes. Pure data parallel:
batch sharded 4-per-core across 8 NeuronCores, weights replicated.

Key algebraic restructurings (validated in numpy to rel-err ~2.5e-6):
  * dist^2 via matmul: d2[i,j] = r2_i + r2_j - 2(x_i x_j + y_i y_j),
    computed exactly-enough with a 3-part bf16 split (K=18 rows, full
    PE streaming rate, ~fp32 precision).
  * adjacency kept as the SIGN matrix s = sign(T - d2) in {-1,+1}
    (exact in bf16); deg comes free from the ACT accum_out of the same
    instruction; adj = (s+1)/2 algebra pushed into the matmuls.
  * mean_i(nadj @ h) collapses: sum_i dinv_i adj_ij dinv_j h_jk =
    sum_j (w dinv)_j h_jk with w = adj @ dinv, so the second GCN layer
    is a matvec, not an [N,N]@[N,256] matmul.
  * u_j > 0 strictly, so u_j relu(q_jk) = relu(u_j q_jk): the weighted
    node-sum of relu(h W1) folds into ACT relu + accum_out.
"""

import sys

if "/opt/trn_rl_repo" not in sys.path:
    sys.path.insert(0, "/opt/trn_rl_repo")

import numpy as np

import concourse.bacc as bacc
import concourse.bass as bass
import concourse.bass_isa as bass_isa
import concourse.tile as tile
from concourse import masks, mybir
from concourse.bass_utils import run_bass_kernel_spmd

F32 = mybir.dt.float32
BF16 = mybir.dt.bfloat16
AF = mybir.ActivationFunctionType

B, N, FEAT = 32, 1024, 7
NCORES = 8
BL = B // NCORES          # batches per core
NT = N // 128             # node tiles
HID = 256
MLP_H = 64
OUT = 8
KFLAT = N * FEAT          # 7168
NKT = KFLAT // 128        # 56


def _threshold() -> float:
    """Smallest fp32 d2 with sqrt_f32(d2) >= 0.3f; then (d2 < T) == (sqrt(d2) < 0.3f)."""
    f3 = np.float32(0.3)
    c = np.float32(f3 * f3)
    for _ in range(200):
        if np.sqrt(c) >= f3:
            c = np.nextafter(c, np.float32(0), dtype=np.float32)
        else:
            break
    while np.sqrt(np.nextafter(c, np.float32(1), dtype=np.float32)) < f3:
        c = np.nextafter(c, np.float32(1), dtype=np.float32)
    return float(np.nextafter(c, np.float32(1), dtype=np.float32))


THRESH = _threshold()

_NC_CACHE = {}


def _build():
    nc = bacc.Bacc("TRN2", target_bir_lowering=False, debug=False)

    x_d = nc.dram_tensor("x", (BL, N, FEAT), F32, kind="ExternalInput")
    w1_d = nc.dram_tensor("W1", (HID, 2), F32, kind="ExternalInput")
    b1_d = nc.dram_tensor("b1", (HID,), F32, kind="ExternalInput")
    w2_d = nc.dram_tensor("W2", (HID, HID), F32, kind="ExternalInput")
    b2_d = nc.dram_tensor("b2", (HID,), F32, kind="ExternalInput")
    wfc_d = nc.dram_tensor("Wfc", (HID, HID), F32, kind="ExternalInput")
    bfc_d = nc.dram_tensor("bfc", (HID,), F32, kind="ExternalInput")
    wg_d = nc.dram_tensor("Wg", (8, 2), F32, kind="ExternalInput")
    bg_d = nc.dram_tensor("bg", (8,), F32, kind="ExternalInput")
    wm0_d = nc.dram_tensor("Wm0", (MLP_H, KFLAT), F32, kind="ExternalInput")
    bm0_d = nc.dram_tensor("bm0", (MLP_H,), F32, kind="ExternalInput")
    wm1_d = nc.dram_tensor("Wm1", (MLP_H, MLP_H), F32, kind="ExternalInput")
    bm1_d = nc.dram_tensor("bm1", (MLP_H,), F32, kind="ExternalInput")
    wp_d = nc.dram_tensor("Wp", (OUT, MLP_H + HID + 8), F32, kind="ExternalInput")
    bp_d = nc.dram_tensor("bp", (OUT,), F32, kind="ExternalInput")
    out_d = nc.dram_tensor("out", (BL, OUT), F32, kind="ExternalOutput")

    with tile.TileContext(nc) as tc:
        _emit(nc, tc, x_d, w1_d, b1_d, w2_d, b2_d, wfc_d, bfc_d, wg_d, bg_d,
              wm0_d, bm0_d, wm1_d, bm1_d, wp_d, bp_d, out_d)
    nc.compile()
    return nc


def _emit(nc, tc, x_d, w1_d, b1_d, w2_d, b2_d, wfc_d, bfc_d, wg_d, bg_d,
          wm0_d, bm0_d, wm1_d, bm1_d, wp_d, bp_d, out_d):
    from contextlib import ExitStack
    ctx = ExitStack()

    const = ctx.enter_context(tc.tile_pool(name="const", bufs=1))
    work = ctx.enter_context(tc.tile_pool(name="work", bufs=2))
    spool = ctx.enter_context(tc.tile_pool(name="spool", bufs=2))
    scratch = ctx.enter_context(tc.tile_pool(name="scratch", bufs=2))

    # ---------------- phase 0: constants, weights, input staging ----------------
    ident = const.tile([128, 128], F32)
    masks.make_identity(nc, ident[:])
    identb = const.tile([128, 128], BF16)
    masks.make_identity(nc, identb[:])

    ones3 = const.tile([3, N], BF16)
    nc.vector.memset(ones3[:], 1.0)
    onesf = const.tile([1, BL], F32)
    nc.vector.memset(onesf[:], 1.0)
    tbias = const.tile([128, 1], F32)
    nc.vector.memset(tbias[:], THRESH)
    b512 = const.tile([128, 1], F32)
    nc.vector.memset(b512[:], float(N) / 2.0)

    # node-partition staging of x: X[p, b, t, f] = x[b, t*128+p, f]
    X = const.tile([128, BL, NT, FEAT], F32)
    nc.sync.dma_start(out=X[:], in_=x_d.ap().rearrange("b (t p) f -> p b t f", p=128))

    # xf^T tiles for the MLP branch: xfT[p, kt, b] = xf[b, kt*128+p]
    xfT = const.tile([128, NKT, BL], F32)
    xf_flat = x_d.ap().rearrange("b n f -> b (n f)")
    for bb in range(BL):
        nc.sync.dma_start(
            out=xfT[:, :, bb:bb + 1],
            in_=xf_flat[bb:bb + 1, :].rearrange("b (kt p) -> p kt b", p=128),
        )

    # natural-layout weights
    wm0nat = const.tile([MLP_H, NKT, 128], F32)
    nc.sync.dma_start(out=wm0nat[:], in_=wm0_d.ap().rearrange("m (kt f) -> m kt f", f=128))
    w2nat = const.tile([128, 2, HID], F32)
    nc.sync.dma_start(out=w2nat[:], in_=w2_d.ap().rearrange("(mt p) k -> p mt k", p=128))
    wfcnat = const.tile([128, 2, HID], F32)
    nc.sync.dma_start(out=wfcnat[:], in_=wfc_d.ap().rearrange("(mt p) k -> p mt k", p=128))
    wm1nat = const.tile([MLP_H, MLP_H], F32)
    nc.sync.dma_start(out=wm1nat[:], in_=wm1_d.ap())

    # biases
    b2np = const.tile([128, 2], F32)
    nc.sync.dma_start(out=b2np[:], in_=b2_d.ap().rearrange("(mt p) -> p mt", p=128))
    bfcnp = const.tile([128, 2], F32)
    nc.sync.dma_start(out=bfcnp[:], in_=bfc_d.ap().rearrange("(mt p) -> p mt", p=128))
    bm0np = const.tile([MLP_H, 1], F32)
    nc.sync.dma_start(out=bm0np[:], in_=bm0_d.ap().rearrange("(p o) -> p o", o=1))
    bm1np = const.tile([MLP_H, 1], F32)
    nc.sync.dma_start(out=bm1np[:], in_=bm1_d.ap().rearrange("(p o) -> p o", o=1))

    # W-side of the q matmul: rows [W1x; W1y; b1], split 2-part
    wside = const.tile([3, HID], F32)
    nc.sync.dma_start(out=wside[0:2, :], in_=w1_d.ap().rearrange("h i -> i h"))
    nc.sync.dma_start(out=wside[2:3, :], in_=b1_d.ap().rearrange("(o h) -> o h", o=1))
    wh = const.tile([3, HID], BF16)
    nc.vector.tensor_copy(out=wh[:], in_=wside[:])
    wlf = const.tile([3, HID], F32)
    nc.vector.tensor_tensor(out=wlf[:], in0=wside[:], in1=wh[:], op=mybir.AluOpType.subtract)
    wl = const.tile([3, HID], BF16)
    nc.vector.tensor_copy(out=wl[:], in_=wlf[:])
    wq9 = const.tile([9, HID], BF16)
    nc.sync.dma_start(out=wq9[0:3, :], in_=wh[:])
    nc.sync.dma_start(out=wq9[3:6, :], in_=wl[:])
    nc.sync.dma_start(out=wq9[6:9, :], in_=wh[:])

    # glo weights: WgTe rows [Wg^T (2); bg]
    wgte = const.tile([3, 8], F32)
    nc.sync.dma_start(out=wgte[0:2, :], in_=wg_d.ap().rearrange("o i -> i o"))
    nc.sync.dma_start(out=wgte[2:3, :], in_=bg_d.ap().rearrange("(o h) -> o h", o=1))

    # Wp^T with permuted row order: tile0 = [cols 0:64 (xf), cols 320:328 (glo), bp],
    # tile1 = cols 64:192 (gcn lo), tile2 = cols 192:320 (gcn hi)
    wpte0 = const.tile([73, 8], F32)
    nc.sync.dma_start(out=wpte0[0:64, :], in_=wp_d.ap()[:, 0:64].rearrange("o k -> k o"))
    nc.sync.dma_start(out=wpte0[64:72, :], in_=wp_d.ap()[:, 320:328].rearrange("o k -> k o"))
    nc.sync.dma_start(out=wpte0[72:73, :], in_=bp_d.ap().rearrange("(o h) -> o h", o=1))
    wpt1 = const.tile([128, 8], F32)
    nc.sync.dma_start(out=wpt1[:], in_=wp_d.ap()[:, 64:192].rearrange("o k -> k o"))
    wpt2 = const.tile([128, 8], F32)
    nc.sync.dma_start(out=wpt2[:], in_=wp_d.ap()[:, 192:320].rearrange("o k -> k o"))

    # node-free staging for the d2 operands: PP rows
    # [px b0..3, py b0..3, r2 b0..3, qx=-2px b0..3, qy=-2py b0..3]
    # (engine ops only ever touch partition base 0; row placement via DMA)
    PP = const.tile([20, N], F32)
    nc.sync.dma_start(out=PP[0:4, :], in_=x_d.ap()[:, :, 1:2].rearrange("b n o -> (b o) n"))
    nc.sync.dma_start(out=PP[4:8, :], in_=x_d.ap()[:, :, 2:3].rearrange("b n o -> (b o) n"))
    pxy4 = const.tile([4, 2 * N], F32)
    nc.sync.dma_start(out=pxy4[:, 0:N], in_=x_d.ap()[:, :, 1:2].rearrange("b n o -> (b o) n"))
    nc.sync.dma_start(out=pxy4[:, N:2 * N], in_=x_d.ap()[:, :, 2:3].rearrange("b n o -> (b o) n"))
    sq4 = const.tile([4, 2 * N], F32)
    nc.vector.tensor_tensor(out=sq4[:], in0=pxy4[:], in1=pxy4[:], op=mybir.AluOpType.mult)
    r24 = const.tile([4, N], F32)
    nc.vector.tensor_tensor(out=r24[:], in0=sq4[:, 0:N], in1=sq4[:, N:2 * N], op=mybir.AluOpType.add)
    nc.sync.dma_start(out=PP[8:12, :], in_=r24[:])
    QQ = const.tile([8, N], F32)
    nc.vector.tensor_scalar_mul(out=QQ[:], in0=PP[0:8, :], scalar1=-2.0)
    nc.sync.dma_start(out=PP[12:20, :], in_=QQ[:])

    # 3-part bf16 split of PP into H1/H2/H3
    H1 = const.tile([20, N], BF16)
    H2 = const.tile([20, N], BF16)
    H3 = const.tile([20, N], BF16)
    nc.vector.tensor_copy(out=H1[:], in_=PP[:])
    D1 = const.tile([20, N], F32)
    nc.vector.tensor_tensor(out=D1[:], in0=PP[:], in1=H1[:], op=mybir.AluOpType.subtract)
    nc.vector.tensor_copy(out=H2[:], in_=D1[:])
    D2 = const.tile([20, N], F32)
    nc.vector.tensor_tensor(out=D2[:], in0=D1[:], in1=H2[:], op=mybir.AluOpType.subtract)
    nc.vector.tensor_copy(out=H3[:], in_=D2[:])
    HPARTS = (H1, H2, H3)

    # --- weight transposes (PE) + the whole MLP branch, in an early PSUM pool ---
    with tc.tile_pool(name="ph0ps", bufs=2, space="PSUM") as ph0ps:
        # Wm0^T [128, kt, 64]
        wm0T = const.tile([128, NKT, MLP_H], F32)
        for grp in range(7):  # 8 transposes per PSUM bank
            pt = ph0ps.tile([128, 512], F32, tag="tps0")
            for j in range(8):
                kt = grp * 8 + j
                nc.tensor.transpose(pt[:, j * 64:(j + 1) * 64], wm0nat[:, kt, :], ident[:MLP_H, :MLP_H])
            nc.vector.tensor_copy(out=wm0T[:, grp * 8:(grp + 1) * 8, :].rearrange("p a b -> p (a b)"), in_=pt[:])

        # W2^T, Wfc^T [128, kt, 256]
        w2T = const.tile([128, 2, HID], F32)
        wfcT = const.tile([128, 2, HID], F32)
        for (nat, dst) in ((w2nat, w2T), (wfcnat, wfcT)):
            for kt in range(2):
                pt = ph0ps.tile([128, 512], F32, tag="tps0")
                for mt in range(2):
                    nc.tensor.transpose(pt[:, mt * 128:(mt + 1) * 128], nat[:, mt, kt * 128:(kt + 1) * 128], ident[:])
                nc.vector.tensor_copy(out=dst[:, kt, :], in_=pt[:, 0:256])

        # Wm1^T [64, 64]
        wm1T = const.tile([MLP_H, MLP_H], F32)
        pt = ph0ps.tile([128, 512], F32, tag="tps0")
        nc.tensor.transpose(pt[:MLP_H, :MLP_H], wm1nat[:], ident[:MLP_H, :MLP_H])
        nc.vector.tensor_copy(out=wm1T[:], in_=pt[:MLP_H, :MLP_H])

        # MLP branch: mlp2 = relu(relu(xf@Wm0^T + bm0)@Wm1^T + bm1), output [64, 4]
        cat0 = const.tile([128, BL], F32)
        nc.gpsimd.memset(cat0[:], 0.0)
        nc.sync.dma_start(out=cat0[72:73, :], in_=onesf[:])

        m1ps = ph0ps.tile([MLP_H, BL], F32, tag="mlp")
        for kt in range(NKT):
            nc.tensor.matmul(m1ps[:], wm0T[:, kt, :], xfT[:, kt, :],
                             start=(kt == 0), stop=(kt == NKT - 1))
        m1sb = const.tile([MLP_H, BL], F32)
        nc.scalar.activation(out=m1sb[:], in_=m1ps[:], func=AF.Relu, bias=bm0np[:], scale=1.0)
        m2ps = ph0ps.tile([MLP_H, BL], F32, tag="mlp")
        nc.tensor.matmul(m2ps[:], wm1T[:], m1sb[:], start=True, stop=True)
        nc.scalar.activation(out=cat0[0:64, :], in_=m2ps[:], func=AF.Relu, bias=bm1np[:], scale=1.0)

    # ---------------- per-batch GCN pipeline ----------------
    d2pool = ctx.enter_context(tc.tile_pool(name="d2ps", bufs=2, space="PSUM"))
    tpool = ctx.enter_context(tc.tile_pool(name="tps", bufs=1, space="PSUM"))
    atpool = ctx.enter_context(tc.tile_pool(name="atps", bufs=1, space="PSUM"))
    qpool = ctx.enter_context(tc.tile_pool(name="qps", bufs=2, space="PSUM"))

    srow = const.tile([128, BL, NT], F32)      # sign-row accums -> deg
    dmax = const.tile([128, BL, NT], F32)      # per-tile d2 maxima
    c_all = const.tile([128, 2, BL, 2], F32)   # relu accums [p, mt, b, half]

    for b in range(BL):
        # --- L/R operand assembly [18, N] bf16 ---
        # pairs (split-part indices): (1,1),(1,2),(1,3),(2,1),(2,2),(3,1)
        # L x-rows = [h1x,h1x,h1x, h2x,h2x, h3x]; R x-rows = [q1x,q2x,q3x, q1x,q2x, q1x]
        L = spool.tile([18, N], BF16, tag="L")
        R = spool.tile([18, N], BF16, tag="R")
        LPAT = [0, 0, 0, 1, 1, 2]   # split-part index per K-row
        RPAT = [0, 1, 2, 0, 1, 0]
        for (coord, lo) in ((0, 0), (4, 6)):      # px base rows 0-3, py 4-7
            for k in range(6):
                nc.sync.dma_start(out=L[lo + k:lo + k + 1, :],
                                  in_=HPARTS[LPAT[k]][coord + b:coord + b + 1, :])
                nc.sync.dma_start(out=R[lo + k:lo + k + 1, :],
                                  in_=HPARTS[RPAT[k]][12 + coord + b:12 + coord + b + 1, :])
        # r2 terms (rows 8-11 of PP): (r2_i * 1) and (1 * r2_j)
        for k in range(3):
            nc.sync.dma_start(out=L[12 + k:13 + k, :], in_=HPARTS[k][8 + b:9 + b, :])
            nc.sync.dma_start(out=R[15 + k:16 + k, :], in_=HPARTS[k][8 + b:9 + b, :])
        nc.sync.dma_start(out=R[12:15, :], in_=ones3[:])
        nc.sync.dma_start(out=L[15:18, :], in_=ones3[:])

        # --- d2 tiles + sign-compare (ACT, with deg accum) + max (DVE) ---
        s_full = spool.tile([128, NT, N], BF16, tag="s")
        for it in range(NT):
            d2t = d2pool.tile([128, N], F32, tag="d2")
            for hf in range(2):
                nc.tensor.matmul(d2t[:, hf * 512:(hf + 1) * 512],
                                 L[:, it * 128:(it + 1) * 128],
                                 R[:, hf * 512:(hf + 1) * 512],
                                 start=True, stop=True)
            nc.scalar.activation(out=s_full[:, it, :], in_=d2t[:], func=AF.Sign,
                                 bias=tbias[:], scale=-1.0,
                                 accum_out=srow[:, b, it:it + 1])
            nc.vector.reduce_max(out=dmax[:, b, it:it + 1], in_=d2t[:],
                                 axis=mybir.AxisListType.X)

        # --- deg -> dinv (node-partition layout) ---
        dinv = work.tile([128, NT], F32, tag="dinv")
        sq = work.tile([128, NT], F32, tag="sqdeg")
        nc.scalar.activation(out=sq[:], in_=srow[:, b, :], func=AF.Sqrt,
                             bias=b512[:], scale=0.5)
        nc.vector.reciprocal(out=dinv[:], in_=sq[:])

        # --- Zext = 2-part split of [dinv*px, dinv*py, dinv] ---
        zf = work.tile([128, NT, 3], F32, tag="zf")
        nc.vector.tensor_tensor(out=zf[:, :, 0:2], in0=X[:, b, :, 1:3],
                                in1=dinv[:, :, None].to_broadcast((128, NT, 2)),
                                op=mybir.AluOpType.mult)
        nc.vector.tensor_copy(out=zf[:, :, 2:3], in_=dinv[:, :, None])
        zext = work.tile([128, NT, 6], BF16, tag="zext")
        nc.vector.tensor_copy(out=zext[:, :, 0:3], in_=zf[:])
        zlf = work.tile([128, NT, 3], F32, tag="zlf")
        nc.vector.tensor_tensor(out=zlf[:], in0=zf[:], in1=zext[:, :, 0:3],
                                op=mybir.AluOpType.subtract)
        nc.vector.tensor_copy(out=zext[:, :, 3:6], in_=zlf[:])

        # column-sum correction cs = sum_nodes Zf  (all-partitions result)
        zred = work.tile([128, 3], F32, tag="zred")
        nc.vector.tensor_reduce(out=zred[:], in_=zf[:].rearrange("p t c -> p c t"),
                                axis=mybir.AxisListType.X, op=mybir.AluOpType.add)
        cs = work.tile([128, 3], F32, tag="cs")
        nc.gpsimd.partition_all_reduce(cs[:], zred[:], channels=128,
                                       reduce_op=bass_isa.ReduceOp.add)

        # --- t/w pass: out[i-tile] = sum_jt s[jt-tile,i]^T-free @ Zext[jt] ---
        twps = tpool.tile([128, NT, 6], F32, tag="tw")
        for it in range(NT):
            for jt in range(NT):
                nc.tensor.matmul(twps[:, it, :],
                                 s_full[:, jt, it * 128:(it + 1) * 128],
                                 zext[:, jt, :],
                                 start=(jt == 0), stop=(jt == NT - 1))
        tw = work.tile([128, NT, 6], F32, tag="twsb")
        nc.vector.tensor_copy(out=tw[:], in_=twps[:])
        # merge hi/lo and add colsum correction (t_true = (merged + cs)/2; the
        # 0.5 factors are folded into the scalar multipliers below)
        t3 = work.tile([128, NT, 3], F32, tag="t3")
        nc.vector.tensor_tensor(out=t3[:], in0=tw[:, :, 0:3], in1=tw[:, :, 3:6],
                                op=mybir.AluOpType.add)
        nc.vector.tensor_tensor(out=t3[:], in0=t3[:],
                                in1=cs[:, None, :].to_broadcast((128, NT, 3)),
                                op=mybir.AluOpType.add)
        # m1 = t3_w * dinv ; m2 = m1 * dinv ; a_xy = m2*t3_xy*(0.25/N); u = m1*(0.5/N)
        m1 = work.tile([128, NT], F32, tag="m1")
        nc.vector.tensor_tensor(out=m1[:], in0=t3[:, :, 2], in1=dinv[:], op=mybir.AluOpType.mult)
        m2 = work.tile([128, NT], F32, tag="m2")
        nc.vector.tensor_tensor(out=m2[:], in0=m1[:], in1=dinv[:], op=mybir.AluOpType.mult)
        a3 = work.tile([128, NT, 3], F32, tag="a3")
        nc.vector.tensor_tensor(out=a3[:, :, 0:2], in0=t3[:, :, 0:2],
                                in1=m2[:, :, None].to_broadcast((128, NT, 2)),
                                op=mybir.AluOpType.mult)
        nc.vector.tensor_scalar_mul(out=a3[:, :, 0:2], in0=a3[:, :, 0:2], scalar1=0.25 / N)
        nc.vector.tensor_scalar_mul(out=a3[:, :, 2:3], in0=m1[:, :, None], scalar1=0.5 / N)

        # --- a9: 2-part split, rows [Ah, Ah, Al] paired with Wq9 [Wh, Wl, Wh] ---
        a9 = work.tile([128, NT, 9], BF16, tag="a9")
        nc.vector.tensor_copy(out=a9[:, :, 0:3], in_=a3[:])
        nc.vector.tensor_copy(out=a9[:, :, 3:6], in_=a3[:])
        alf = work.tile([128, NT, 3], F32, tag="alf")
        nc.vector.tensor_tensor(out=alf[:], in0=a3[:], in1=a9[:, :, 0:3],
                                op=mybir.AluOpType.subtract)
        nc.vector.tensor_copy(out=a9[:, :, 6:9], in_=alf[:])

        # transpose a9 -> aT [9, N] bf16
        atps = atpool.tile([9, N], BF16, tag="at")
        for it in range(NT):
            nc.tensor.transpose(atps[:, it * 128:(it + 1) * 128], a9[:, it, :], identb[:])
        aT = work.tile([9, N], BF16, tag="aT")
        nc.vector.tensor_copy(out=aT[:], in_=atps[:])

        # --- q' matmul + relu + weighted-node-sum accum (c) ---
        for mt in range(2):
            for hf in range(2):
                qps = qpool.tile([128, 512], F32, tag="q")
                nc.tensor.matmul(qps[:], wq9[:, mt * 128:(mt + 1) * 128],
                                 aT[:, hf * 512:(hf + 1) * 512], start=True, stop=True)
                rl = scratch.tile([128, 512], BF16, tag="rl")
                nc.scalar.activation(out=rl[:], in_=qps[:], func=AF.Relu,
                                     accum_out=c_all[:, mt, b, hf:hf + 1])

    # ---------------- final chain (all batches) ----------------
    cm = const.tile([128, 2, BL], F32)
    nc.vector.tensor_tensor(out=cm[:], in0=c_all[:, :, :, 0], in1=c_all[:, :, :, 1],
                            op=mybir.AluOpType.add)

    g1sb = const.tile([128, 2, BL], F32)
    g2sb = const.tile([128, 2, BL], F32)
    for (wT, csrc, bias_np, dst) in ((w2T, cm, b2np, g1sb), (wfcT, g1sb, bfcnp, g2sb)):
        for mt in range(2):
            gps = qpool.tile([128, 512], F32, tag="q")
            for kt in range(2):
                nc.tensor.matmul(gps[:, 0:BL], wT[:, kt, mt * 128:(mt + 1) * 128],
                                 csrc[:, kt, :], start=(kt == 0), stop=(kt == 1))
            nc.scalar.activation(out=dst[:, mt, :], in_=gps[:, 0:BL], func=AF.Identity,
                                 bias=bias_np[:, mt:mt + 1], scale=1.0)

    # --- glo branch ---
    vsq = const.tile([128, BL, NT, 2], F32)
    nc.vector.tensor_tensor(out=vsq[:], in0=X[:, :, :, 3:5], in1=X[:, :, :, 3:5],
                            op=mybir.AluOpType.mult)
    vs2 = const.tile([128, BL, NT], F32)
    nc.vector.tensor_tensor(out=vs2[:], in0=vsq[:, :, :, 0], in1=vsq[:, :, :, 1],
                            op=mybir.AluOpType.add)
    spd = const.tile([128, BL, NT], F32)
    nc.scalar.activation(out=spd[:], in_=vs2[:], func=AF.Sqrt)
    spr = const.tile([128, BL], F32)
    nc.vector.tensor_reduce(out=spr[:], in_=spd[:], axis=mybir.AxisListType.X,
                            op=mybir.AluOpType.add)
    spsum = const.tile([128, BL], F32)
    nc.gpsimd.partition_all_reduce(spsum[:], spr[:], channels=128,
                                   reduce_op=bass_isa.ReduceOp.add)
    dmr = const.tile([128, BL], F32)
    nc.vector.tensor_reduce(out=dmr[:], in_=dmax[:], axis=mybir.AxisListType.X,
                            op=mybir.AluOpType.max)
    dmx = const.tile([128, BL], F32)
    nc.gpsimd.partition_all_reduce(dmx[:], dmr[:], channels=128,
                                   reduce_op=bass_isa.ReduceOp.max)

    gloin = const.tile([3, BL], F32)
    nc.vector.tensor_scalar_mul(out=gloin[0:1, :], in0=spsum[0:1, :], scalar1=1.0 / N)
    dsq = const.tile([1, BL], F32)
    nc.scalar.activation(out=dsq[:], in_=dmx[0:1, :], func=AF.Sqrt)
    drc = const.tile([1, BL], F32)
    nc.vector.reciprocal(out=drc[:], in_=dsq[:])
    nc.sync.dma_start(out=gloin[1:2, :], in_=drc[:])
    nc.sync.dma_start(out=gloin[2:3, :], in_=onesf[:])

    glops = qpool.tile([8, 512], F32, tag="q")
    nc.tensor.matmul(glops[:, 0:BL], wgte[:], gloin[:], start=True, stop=True)
    nc.scalar.activation(out=cat0[64:72, :], in_=glops[:, 0:BL], func=AF.Relu)

    # --- final projection ---
    ops = qpool.tile([8, 512], F32, tag="q")
    nc.tensor.matmul(ops[:, 0:BL], wpte0[:], cat0[0:73, :], start=True, stop=False)
    nc.tensor.matmul(ops[:, 0:BL], wpt1[:], g2sb[:, 0, :], start=False, stop=False)
    nc.tensor.matmul(ops[:, 0:BL], wpt2[:], g2sb[:, 1, :], start=False, stop=True)
    outsb = const.tile([8, BL], F32)
    nc.vector.tensor_copy(out=outsb[:], in_=ops[:, 0:BL])
    nc.sync.dma_start(out=out_d.ap().rearrange("b o -> o b"), in_=outsb[:])

    ctx.close()


def _get_nc():
    if "nc" not in _NC_CACHE:
        _NC_CACHE["nc"] = _build()
    return _NC_CACHE["nc"]


def _prep_inputs(inputs):
    prepped = {}
    for k, v in inputs.items():
        a = np.asarray(v)
        if a.dtype != np.float32:
            a = a.astype(np.float32)
        prepped[k] = np.ascontiguousarray(a)
    return prepped


def run_sharded(inputs, **kwargs):
    """Build per-core in_maps (batch-sharded x, replicated weights) and run."""
    inputs = _prep_inputs(inputs)
    nc = _get_nc()
    x = inputs["x"]
    in_maps = []
    for c in range(NCORES):
        m = {k: v for k, v in inputs.items() if k != "x"}
        m["x"] = np.ascontiguousarray(x[c * BL:(c + 1) * BL])
        in_maps.append(m)
    res = run_bass_kernel_spmd(nc, in_maps, core_ids=list(range(NCORES)), **kwargs)
    out = np.concatenate([res.results[c]["out"] for c in range(NCORES)], axis=0)
    return out, res


def kernel(**inputs) -> np.ndarray:
    out, _ = run_sharded(inputs)
    return out
